# revision 10
# baseline (speedup 1.0000x reference)
"""Trainium2 Bass kernel for relative-position multi-head attention.

Problem shape (hardcoded): B=2, T=1024, CH=1024, HEADS=16, KC=64, WIN=4.

Sharding v2: core = (batch b, head-quad hq) with b = core//4, hq = core%4.
Each core computes q/k/v projections for its 4 heads over its batch's 1024
tokens, runs attention for those 4 heads, then multiplies its attention
output block (256 channels) by the matching 256 ROWS of Wo, producing a
full-size PARTIAL output [1024, 1024].  The host sums the 4 partials per
batch and adds bo.  There are NO on-device collectives, so each core's
NEFF execution is independent of the other cores' launch times.

Relative-position band trick (unchanged from v1): with T=1024 and window 4,
the relative logits/weights only touch the 9 diagonals |j-i| <= 4 of the
[T, T] score matrix.  The band-add (scores) and band-gather (rel_v
epilogue) go through a small DRAM staging buffer in "diagonal-compact"
layout: value for (j, i) at flat (j+4)*144 + (i-j+4).  Rectangular windows
[128, 136] of the [j, i] plane map to strided-contiguous patterns there,
and unused slots m in [9, 144) are zeroed once so out-of-band window cells
read zeros.

The exp(S) tensor, PV matmul operands and the rel-v gather run in bf16
(scores stay fp32): halves SBUF footprint and band DMA traffic; error
stays ~1e-3 (threshold 2e-2).
"""

import sys

sys.path.insert(0, "/opt/trn_rl_repo")

import math
import numpy as np

import concourse.bass as bass
import concourse.tile as tile
from concourse import mybir
from concourse import bacc
from concourse.bass_utils import run_bass_kernel_spmd
from concourse.masks import make_identity

# ---------------------------------------------------------------- constants
B, T, CH, HEADS, KC, WIN = 2, 1024, 1024, 16, 64, 4
NCORES = 8
HPC = 4                        # heads per core (one quad)
DPC = HPC * KC                 # channels per core = 256
M9 = 2 * WIN + 1               # 9 diagonals
WSLOT = 144                    # diag-compact row stride (> 135 garbage range)
WBND = 136                     # band window width (i in [j0-4, j0+132))
EXPW = 8 * 1024 + 8            # per-unit exp(S) tensor width (+8 pad cols)
F32 = mybir.dt.float32
F32R = mybir.dt.float32r
BF16 = mybir.dt.bfloat16
AF = mybir.ActivationFunctionType

_CACHE = {}


# ---------------------------------------------------------------- program
def build_program():
    nc = bacc.Bacc("TRN2", target_bir_lowering=False, debug=False,
                   num_devices=NCORES)

    xT = nc.dram_tensor("xT", [CH, T], F32R, kind="ExternalInput")
    cT = nc.dram_tensor("cT", [CH, T], F32R, kind="ExternalInput")
    # wq/wk/wv[p, 256*d8 + j] = W[128*d8 + p, 256*hq + j]  (q pre-scaled)
    wq = nc.dram_tensor("wq", [128, 2048], F32R, kind="ExternalInput")
    wk = nc.dram_tensor("wk", [128, 2048], F32R, kind="ExternalInput")
    wv = nc.dram_tensor("wv", [128, 2048], F32R, kind="ExternalInput")
    # wo[p, 1024*g + ch] = Wo[256*hq + 128*g + p, ch]
    wo = nc.dram_tensor("wo", [128, 2048], F32R, kind="ExternalInput")
    # bqkv columns: [bq_g0, bq_g1, bk_g0, bk_g1, bv_g0, bv_g1]
    bqkv = nc.dram_tensor("bqkv", [128, 6], F32, kind="ExternalInput")
    erk = nc.dram_tensor("erk", [128, M9], F32R, kind="ExternalInput")
    erv = nc.dram_tensor("erv", [M9, KC + 1], BF16, kind="ExternalInput")
    zros = nc.dram_tensor("zros", [128, 1164], F32, kind="ExternalInput")
    outP = nc.dram_tensor("outP", [T, CH], F32, kind="ExternalOutput")

    with tile.TileContext(nc) as tc:
        with (
            tc.tile_pool(name="const", bufs=1) as cpool,
            tc.tile_pool(name="persist", bufs=1) as ppool,
            tc.tile_pool(name="dram", bufs=1, space="DRAM") as dpool,
        ):
            # ---------------- constants / weights to SBUF (contiguous DMAs)
            wq_sb = cpool.tile([128, 2048], F32R, name="wq_sb")
            wk_sb = cpool.tile([128, 2048], F32R, name="wk_sb")
            wv_sb = cpool.tile([128, 2048], F32R, name="wv_sb")
            wo_sb = cpool.tile([128, 2048], F32R, name="wo_sb")
            nc.scalar.dma_start(wq_sb[:], wq[:])
            nc.scalar.dma_start(wk_sb[:], wk[:])
            nc.scalar.dma_start(wv_sb[:], wv[:])
            nc.scalar.dma_start(wo_sb[:], wo[:])
            bqkv_sb = cpool.tile([128, 6], F32, name="bqkv_sb")
            nc.scalar.dma_start(bqkv_sb[:], bqkv[:])
            erk_sb = cpool.tile([128, M9], F32R, name="erk_sb")
            nc.scalar.dma_start(erk_sb[:], erk[:])
            erv_sb = cpool.tile([M9, KC + 1], BF16, name="erv_sb")
            nc.scalar.dma_start(erv_sb[:], erv[:])
            ident = cpool.tile([128, 128], F32, name="ident")
            make_identity(nc, ident[:])

            # persistent activations: [128 rows = 2 heads x 64, 1024*g + tok]
            qT_sb = ppool.tile([128, 2048], F32R, name="qT_sb")
            kT_sb = ppool.tile([128, 2048], F32R, name="kT_sb")
            vT_sb = ppool.tile([128, 2048], F32, name="vT_sb")
            OT_sb = ppool.tile([128, 2048], F32R, name="OT_sb")
            # v in [j, d] layout + ones column, per unit (head): [128, 8*65]
            vju = [ppool.tile([128, 8 * (KC + 1)], BF16, name=f"vju{u}")
                   for u in range(4)]

            # DRAM staging for the diagonal band, one pair per unit
            rd_d = [dpool.tile([128 * 1164], F32, name=f"rd{u}")
                    for u in range(4)]
            gd_d = [dpool.tile([(T + 8) * WSLOT], BF16, name=f"gd{u}")
                    for u in range(4)]

            for u in range(4):
                # zero Rd (slots >= 9 and j-header/tail must read 0)
                flat = rd_d[u][:].rearrange("(r c) -> r c", c=1164)
                nc.sync.dma_start(flat[:], zros[:])
                # Gd: only the j-invalid edge rows must be zero
                flatg = gd_d[u][:].rearrange("(r c) -> r c", c=WSLOT)
                nc.sync.dma_start(flatg[0:4, :],
                                  zros[:4, :WSLOT // 2].bitcast(BF16))
                nc.sync.dma_start(flatg[T + 4:T + 8, :],
                                  zros[:4, :WSLOT // 2].bitcast(BF16))
            # ones columns of vju (strided memset per unit)
            for u in range(4):
                dst = bass.AP(vju[u].tensor, KC,
                              [[8 * (KC + 1), 128], [KC + 1, 8]])
                nc.vector.memset(dst, 1.0)

            # ---------------- phase A: QKV projections (transposed layouts)
            # q/k/v run CONCURRENTLY per token-half: 6 PSUM accumulators
            # (3 proj x 2 groups), x on the gpsimd queue, c on sync.
            with (
                tc.tile_pool(name="xin", bufs=16) as xpool,
                tc.tile_pool(name="qkvps", bufs=1, space="PSUM") as qkvps,
                tc.tile_pool(name="tps", bufs=2, space="PSUM") as tpps,
            ):
                xts, cts = [], []
                for d8 in range(8):
                    tx = xpool.tile([128, T], F32R, tag="xt")
                    nc.gpsimd.dma_start(tx[:], xT[d8 * 128:(d8 + 1) * 128, :])
                    xts.append(tx)
                    tcc = xpool.tile([128, T], F32R, tag="xt")
                    nc.sync.dma_start(tcc[:], cT[d8 * 128:(d8 + 1) * 128, :])
                    cts.append(tcc)
                projs = (("q", wq_sb, xts, qT_sb, 0),
                         ("k", wk_sb, cts, kT_sb, 2),
                         ("v", wv_sb, cts, vT_sb, 4))
                for it in range(2):
                    acc = {nm: [qkvps.tile([128, 512], F32, tag=f"{nm}{g}",
                                           name=f"{nm}p{g}_{it}")
                                for g in range(2)]
                           for nm, *_ in projs}
                    for d8 in range(8):
                        for nm, w_sb, src, _, _ in projs:
                            for g in range(2):
                                nc.tensor.matmul(
                                    acc[nm][g][:],
                                    w_sb[:, 256 * d8 + 128 * g:
                                         256 * d8 + 128 * (g + 1)],
                                    src[d8][:, 512 * it:512 * (it + 1)],
                                    start=(d8 == 0), stop=(d8 == 7))
                    for nm, _, _, dstT, bcol in projs:
                        for g in range(2):
                            nc.vector.tensor_scalar_add(
                                dstT[:, 1024 * g + 512 * it:
                                     1024 * g + 512 * (it + 1)],
                                acc[nm][g][:], bqkv_sb[:, bcol + g:bcol + g + 1])

                # transpose v to [j, d] per unit; ones col via memset
                for u in range(4):
                    g, hb = u // 2, 64 * (u % 2)
                    for jc in range(8):
                        tp = tpps.tile([128, KC], F32, tag="tp")
                        nc.tensor.transpose(
                            tp[:],
                            vT_sb[hb:hb + 64,
                                  1024 * g + 128 * jc:1024 * g + 128 * (jc + 1)],
                            ident[hb:hb + 64, hb:hb + 64])
                        nc.vector.tensor_copy(
                            vju[u][:, 65 * jc:65 * jc + 64], tp[:])

            # ---------------- phase B: attention, units interleaved in pairs
            # so every engine always has independent work from the sibling
            # unit (the per-jt PE->DVE->ACT->PE chain is latency-bound).
            with (
                tc.tile_pool(name="spool", bufs=2, space="PSUM") as spool,
                tc.tile_pool(name="opool", bufs=2, space="PSUM") as opool,
                tc.tile_pool(name="bnd", bufs=2) as bndpool,
                tc.tile_pool(name="exps", bufs=2) as exppool,
                tc.tile_pool(name="misc", bufs=2) as mpool,
            ):
                def unit_prolog(u):
                    g, hb = u // 2, 64 * (u % 2)
                    cb = 1024 * g
                    # R^T[t, i] = sum_d erk[t, d] * qs[d, i]   -> [9, 1024]
                    rp = spool.tile([M9, T], F32, tag="sps", name=f"rp{u}")
                    for s in range(2):
                        nc.tensor.matmul(
                            rp[:, 512 * s:512 * (s + 1)],
                            erk_sb[hb:hb + 64, :],
                            qT_sb[hb:hb + 64, cb + 512 * s:cb + 512 * (s + 1)],
                            start=True, stop=True)
                    r_sb = mpool.tile([M9, T], F32, tag="r_sb")
                    nc.vector.tensor_copy(r_sb[:], rp[:])
                    # staircase write: r_sb[t, i] -> Rd[(i+t)*144 + (8-t)]
                    dst = bass.AP(rd_d[u].tensor, 8,
                                  [[WSLOT - 1, M9], [WSLOT, T]])
                    nc.sync.dma_start(dst, r_sb[:])
                    # all 8 band windows in one DMA: Bnd[p, 136*jt + c]
                    bnd = bndpool.tile([128, 8 * WBND], F32, tag="bnd")
                    srcb = bass.AP(rd_d[u].tensor, 4 * WSLOT,
                                   [[WSLOT - 1, 128], [128 * WSLOT, 8],
                                    [1, WBND]])
                    nc.sync.dma_start(
                        bnd[:].rearrange("p (j c) -> p j c", c=WBND), srcb)
                    expt = exppool.tile([128, EXPW], BF16, tag="expt")
                    op = [opool.tile([KC + 1, 512], F32, tag=f"ops{s}",
                                     name=f"ops{s}_{u}") for s in range(2)]
                    return dict(g=g, hb=hb, cb=cb, bnd=bnd, expt=expt, op=op)

                def unit_jt(u, st, jt):
                    hb, cb = st["hb"], st["cb"]
                    j0 = 128 * jt
                    sp = spool.tile([128, T], F32, tag="sps")
                    for s in range(2):
                        nc.tensor.matmul(
                            sp[:, 512 * s:512 * (s + 1)],
                            kT_sb[hb:hb + 64, cb + j0:cb + j0 + 128],
                            qT_sb[hb:hb + 64,
                                  cb + 512 * s:cb + 512 * (s + 1)],
                            start=True, stop=True)
                    # band add: window i in [j0-4, j0+132), clipped
                    a = max(0, j0 - 4)
                    e = min(T, j0 + 132)
                    s0 = a - (j0 - 4)
                    nc.vector.tensor_add(
                        sp[:, a:e], sp[:, a:e],
                        st["bnd"][:, WBND * jt + s0:WBND * jt + s0 + (e - a)])
                    ecol = 1024 * jt
                    nc.scalar.activation(st["expt"][:, ecol:ecol + T], sp[:],
                                         AF.Exp)
                    # PV + colsum (ones column fused in vju)
                    for s in range(2):
                        nc.tensor.matmul(
                            st["op"][s][:],
                            vju[u][:, 65 * jt:65 * (jt + 1)],
                            st["expt"][:, ecol + 512 * s:ecol + 512 * (s + 1)],
                            start=(jt == 0), stop=False)

                def unit_epilog(u, st):
                    hb, cb, expt, op = st["hb"], st["cb"], st["expt"], st["op"]
                    # band windows of exp(S) -> Gd (2 DMAs: jt=0, jt=1..7)
                    dst0 = bass.AP(gd_d[u].tensor, 4 * WSLOT + 4,
                                   [[WSLOT - 1, 128], [1, 132]])
                    nc.scalar.dma_start(dst0, expt[:, 0:132])
                    dst17 = bass.AP(gd_d[u].tensor, 132 * WSLOT,
                                    [[WSLOT - 1, 128], [128 * WSLOT, 7],
                                     [1, WBND]])
                    src17 = bass.AP(expt.tensor, 1148,
                                    [[EXPW, 128], [1024 + 128, 7], [1, WBND]])
                    nc.scalar.dma_start(dst17, src17)
                    # gather the 9 diagonals of exp(S): G9[t, i]
                    g9 = mpool.tile([M9, T], BF16, tag="g9")
                    srcg = bass.AP(gd_d[u].tensor, 8,
                                   [[WSLOT - 1, M9], [WSLOT, T]])
                    nc.scalar.dma_start(g9[:], srcg)
                    for s in range(2):
                        nc.tensor.matmul(
                            op[s][:], erv_sb[:],
                            g9[:, 512 * s:512 * (s + 1)],
                            start=False, stop=True)
                    # normalize by colsum (row KC) and write to OT
                    cs1 = mpool.tile([1, T], F32, tag="cs1")
                    rcp64 = mpool.tile([64, T], F32, tag="rcp64")
                    for s in range(2):
                        nc.vector.tensor_copy(cs1[:, 512 * s:512 * (s + 1)],
                                              op[s][KC:KC + 1, :])
                    nc.vector.reciprocal_approx_fast(cs1[:], cs1[:])
                    nc.gpsimd.partition_broadcast(rcp64[:], cs1[:])
                    for s in range(2):
                        nc.vector.tensor_mul(
                            OT_sb[hb:hb + 64, cb + 512 * s:cb + 512 * (s + 1)],
                            op[s][0:KC, :], rcp64[:, 512 * s:512 * (s + 1)])

                for pair in range(2):
                    units = (2 * pair, 2 * pair + 1)
                    sts = {u: unit_prolog(u) for u in units}
                    for jt in range(8):
                        for u in units:
                            unit_jt(u, sts[u], jt)
                    for u in units:
                        unit_epilog(u, sts[u])

            # ---------------- phase C: partial output projection (no bias;
            # host sums the 4 per-batch partials and adds bo)
            with (
                tc.tile_pool(name="fps", bufs=4, space="PSUM") as fpool,
                tc.tile_pool(name="osb", bufs=4) as opool2,
            ):
                engs = (nc.sync, nc.gpsimd, nc.scalar)
                for tb in range(8):
                    for s in range(2):
                        fp = fpool.tile([128, 512], F32, tag="fp")
                        for g in range(2):
                            nc.tensor.matmul(
                                fp[:],
                                OT_sb[:, 1024 * g + 128 * tb:
                                      1024 * g + 128 * (tb + 1)],
                                wo_sb[:, 1024 * g + 512 * s:
                                      1024 * g + 512 * (s + 1)],
                                start=(g == 0), stop=(g == 1))
                        ot = opool2.tile([128, 512], F32, tag="osb")
                        nc.vector.tensor_copy(ot[:], fp[:])
                        engs[(tb * 2 + s) % 3].dma_start(
                            outP[128 * tb:128 * (tb + 1),
                                 512 * s:512 * (s + 1)], ot[:])

    nc.compile()
    return nc


# ---------------------------------------------------------------- host side
def _prep_inputs(x, c, Wq, bq, Wk, bk, Wv, bv, Wo, bo, emb_rel_k, emb_rel_v):
    import ml_dtypes
    scale = 1.0 / math.sqrt(KC)
    xT = [np.ascontiguousarray(x[b].T.astype(np.float32)) for b in range(B)]
    cT = [np.ascontiguousarray(c[b].T.astype(np.float32)) for b in range(B)]
    Wq_s = (Wq * scale).astype(np.float32)
    bq_s = (bq * scale).astype(np.float32)
    Wk_f = Wk.astype(np.float32)
    Wv_f = Wv.astype(np.float32)
    Wo_f = Wo.astype(np.float32)
    bk_f = bk.astype(np.float32)
    bv_f = bv.astype(np.float32)
    erk2 = np.ascontiguousarray(
        np.concatenate([emb_rel_k[0].T, emb_rel_k[0].T], axis=0)
    ).astype(np.float32)                                       # [128, 9]
    erv_p = np.concatenate(
        [emb_rel_v[0], np.zeros((M9, 1), np.float32)],
        axis=1).astype(ml_dtypes.bfloat16)                     # [9, 65]
    zros = np.zeros((128, 1164), np.float32)

    def chunk8(w):  # [1024, 256] -> [128, 8*256] with d8-major free dim
        return np.ascontiguousarray(
            w.reshape(8, 128, 256).transpose(1, 0, 2).reshape(128, 2048))

    in_maps = []
    for cix in range(NCORES):
        b, hq = divmod(cix, 4)
        sl = slice(DPC * hq, DPC * (hq + 1))
        wo_p = np.ascontiguousarray(
            Wo_f[sl, :].reshape(2, 128, CH).transpose(1, 0, 2).reshape(
                128, 2048))
        bqkv = np.stack([
            bq_s[sl][:128], bq_s[sl][128:],
            bk_f[sl][:128], bk_f[sl][128:],
            bv_f[sl][:128], bv_f[sl][128:]], axis=1)
        in_maps.append({
            "xT": xT[b], "cT": cT[b],
            "wq": chunk8(Wq_s[:, sl]),
            "wk": chunk8(Wk_f[:, sl]),
            "wv": chunk8(Wv_f[:, sl]),
            "wo": wo_p,
            "bqkv": np.ascontiguousarray(bqkv),
            "erk": erk2,
            "erv": erv_p,
            "zros": zros,
        })
    return in_maps


def _numpy_fallback(x, c, mask, Wq, bq, Wk, bk, Wv, bv, Wo, bo,
                    emb_rel_k, emb_rel_v):
    # general-mask reference path (never taken for the spec'd all-ones mask)
    NI = B * T
    q = (x.reshape(NI, CH) @ Wq + bq).reshape(B, T, HEADS, KC).transpose(0, 2, 1, 3)
    k = (c.reshape(NI, CH) @ Wk + bk).reshape(B, T, HEADS, KC).transpose(0, 2, 1, 3)
    v = (c.reshape(NI, CH) @ Wv + bv).reshape(B, T, HEADS, KC).transpose(0, 2, 1, 3)
    qs = q / math.sqrt(KC)
    scores = np.einsum("bhtd,bhsd->bhts", qs, k)
    idx_j = np.arange(T)[None, :] - np.arange(T)[:, None] + WIN  # j - i + 4
    band = (idx_j >= 0) & (idx_j <= 2 * WIN)
    rel = np.einsum("bhtd,md->bhtm", qs, emb_rel_k[0])  # [B,H,T,9]
    bias = np.zeros((B, HEADS, T, T), np.float32)
    ii, jj = np.nonzero(band)
    bias[:, :, ii, jj] = rel[:, :, ii, idx_j[ii, jj]]
    scores = scores + bias
    scores = np.where(mask == 0, np.float32(1e-4), scores)
    scores -= scores.max(axis=-1, keepdims=True)
    p = np.exp(scores)
    p /= p.sum(axis=-1, keepdims=True)
    out = np.einsum("bhts,bhsd->bhtd", p, v)
    relw = np.zeros((B, HEADS, T, M9), np.float32)
    relw[:, :, ii, idx_j[ii, jj]] = p[:, :, ii, jj]
    out = out + np.einsum("bhtm,md->bhtd", relw, emb_rel_v[0])
    out = out.transpose(0, 2, 1, 3).reshape(NI, CH)
    return (out @ Wo + bo).reshape(B, T, CH).astype(np.float32)


def kernel(x, c, mask, Wq, bq, Wk, bk, Wv, bv, Wo, bo, emb_rel_k, emb_rel_v,
           _collect=None):
    x = np.asarray(x); c = np.asarray(c); mask = np.asarray(mask)
    args = [np.asarray(a) for a in
            (Wq, bq, Wk, bk, Wv, bv, Wo, bo, emb_rel_k, emb_rel_v)]
    if not np.all(mask):
        return _numpy_fallback(x, c, mask, *args)

    if "nc" not in _CACHE:
        _CACHE["nc"] = build_program()
    nc = _CACHE["nc"]

    in_maps = _prep_inputs(x, c, *args)
    res = run_bass_kernel_spmd(nc, in_maps, core_ids=list(range(NCORES)))
    if _collect is not None:
        _collect.append(res)
    bo_f = args[7].astype(np.float32)
    out = np.empty((B, T, CH), np.float32)
    for b in range(B):
        acc = res.results[4 * b]["outP"].copy()
        for hq in range(1, 4):
            acc += res.results[4 * b + hq]["outP"]
        out[b] = acc + bo_f
    return out
